# revision 69
# baseline (speedup 1.0000x reference)
"""Multi-head attention (B=4, S=1500, D=1024, H=16) on 8 TRN2 NeuronCores.

Sharding: (batch, head-half) -> core c = 2*b + h; each core computes the full
attention for batch b, heads h*8..h*8+7, plus its partial contribution to the
output projection (contraction over its 512 features). Host sums the two
partials per batch, adds bo + Wo@bv (the v-bias term commutes through the
output projection), and stacks.

Numerics (end-to-end max-rel err ~1.5e-2, dominated by one fp8 cast of q):
  q/k/v projections run on the PE in fp8e4 + DoubleRow perf mode (0.5
  cycles/row, 256 contraction rows/instr) using a 3-term residual split
  x@W ~= x8@W8 + x8@Wr + xr@W8 (x8/W8 fp8 casts, xr/Wr fp8 casts of the
  cast residuals; the dropped xr@Wr term is ~0.07%). Wq/Wk/Wv are
  host-scaled by 32 so their fp8 mantissas are in range; the scale is
  compensated in the exp scale (q and k both carry 32x) and in the v
  staging (x 1/32).
  scores S^T[k,q] per head: one DoubleRow matmul with lhsT groups =
  (k_hi, k_lo) fp8 (exact k) and rhs groups = (q8, q8) (one fp8 cast of q:
  the only significant error). exp on ACT (the critical engine, ~138us)
  -> P^T in fp16.
  U: WVn[q, 65] += pt[k, qchunk].T @ v_aug[k, 65] in fp16 (full output
  lanes, N=65). The 12 q-tile accumulators pack into 2 PSUM banks via DVE
  memset + start=False accumulation. v's 65th ones-column gives softmax
  denominators. normalize: per-partition reciprocal + TSP -> WV[q,f] fp16;
  WV -> wvT via XBAR DMA-transpose (no PE/DVE cost). out-projection fp16.

Schedule: HWDGE dispatch (625ns each) and the DMA engines serialize
globally, so input DMAs are strictly priority-ordered on the SP queue
(packed pair-0 q/k weights, x seq-chunks, then t1-3 weights + wo); the
first three score tiles (h0, kt0-2) are computed and exp'd per-512-col
q-chunk so ACT starts as soon as x chunk 0 lands. Projection fillers for
pairs 1-3 are 128-col units (PSUM-bank-safe; each borrows a big3 slot for
~0.85us, under the 1.435us exp-cadence slack): q units fill the pair's odd
window, k units run just-in-time in the even window at slot = kt - 2; the
k fp8 hi/lo split runs on the otherwise-idle Pool engine off-borrow via a
f32 scratch copied in-borrow on DVE. v projection borrows the uacc PSUM
banks during head 0's window; U(h) runs during head h+1's window; PSUM =
2x [128,1536] (S^T double-buffer + filler accumulators) + 2x [128,512]
accumulator banks. Tail: U(7) drains into a held big3 slot qt-outer while
the first five out-proj units pre-open fc0-2 accumulations; per-qt
normalize + pair-3 transposes (alternating HWDGE queues) trickle out, then
24 (sq, ch) units rotate 8 psum regions (fc chain + ACT/DVE-alternating
copyout into the contiguous yp staging buffer), and y leaves in 5 batched
DMAs.
"""

import os
import numpy as np

N_STATE = 1024
B = 4
S = 1500
F = 512            # features per core (8 heads x 64)
NKT = 12           # seq k-tiles of 128, last = 92
KPAD = 1536
VBLK = 520         # 8 heads * 65 cols (64 d + ones) per seq tile in v_sb
SLOTW = 80         # uacc accumulator slot stride (f32 cols)
QCH = [(0, 512), (512, 512), (1024, 476)]
QCH6 = [(0, 256), (256, 256), (512, 256), (768, 256), (1024, 256), (1280, 220)]
# matmul PSUM outputs must not cross 512-elem (2KB) bank boundaries, so
# filler-unit grids stay 128/256-aligned
QCH12 = [(i * 128, 128) for i in range(11)] + [(1408, 92)]
SCALE = 0.125      # 1/sqrt(64)
WSCALE = 32.0      # host scaling of Wq/Wk/Wv before fp8 cast
EXP_SCALE = SCALE / (WSCALE * WSCALE)
NCORES = 8

_CACHE = {}
LAST_RESULTS = None
LABELS = []


def _build():
    import concourse.mybir as mybir
    import concourse.tile as tile
    from concourse import bacc

    f32 = mybir.dt.float32
    fp16 = mybir.dt.float16
    fp8 = mybir.dt.float8e4
    Exp = mybir.ActivationFunctionType.Exp
    Copy = mybir.ActivationFunctionType.Copy
    DR = mybir.MatmulPerfMode.DoubleRow

    nc = bacc.Bacc("TRN2", target_bir_lowering=False, debug=False,
                   num_devices=NCORES)

    x8d = nc.dram_tensor("x8d", [128, 4, 2, KPAD], fp8,
                         kind="ExternalInput").ap()
    xrd = nc.dram_tensor("xrd", [128, 4, 2, KPAD], fp8,
                         kind="ExternalInput").ap()
    wq8 = nc.dram_tensor("wq8", [4, 128, 4, 2, 128], fp8,
                         kind="ExternalInput").ap()
    wqr = nc.dram_tensor("wqr", [4, 128, 4, 2, 128], fp8,
                         kind="ExternalInput").ap()
    wk8 = nc.dram_tensor("wk8", [4, 128, 4, 2, 128], fp8,
                         kind="ExternalInput").ap()
    wkr = nc.dram_tensor("wkr", [4, 128, 4, 2, 128], fp8,
                         kind="ExternalInput").ap()
    wqp = nc.dram_tensor("wqp", [128, 2, 4, 2, 128], fp8,
                         kind="ExternalInput").ap()
    wkp = nc.dram_tensor("wkp", [128, 2, 4, 2, 128], fp8,
                         kind="ExternalInput").ap()
    wv8 = nc.dram_tensor("wv8", [128, 4, 2, F], fp8, kind="ExternalInput").ap()
    wvr = nc.dram_tensor("wvr", [128, 4, 2, F], fp8, kind="ExternalInput").ap()
    wod = nc.dram_tensor("wod", [128, 4, N_STATE], fp16,
                         kind="ExternalInput").ap()
    iden = nc.dram_tensor("iden", [128, 128], fp16,
                          kind="ExternalInput").ap()
    bqh = nc.dram_tensor("bqh", [128, 4], f32, kind="ExternalInput").ap()
    y = nc.dram_tensor("y", [S, N_STATE], fp16, kind="ExternalOutput").ap()

    mm = nc.tensor.matmul

    def lbl(s):
        LABELS.append((int(nc.get_next_instruction_name()[2:]), s))

    with tile.TileContext(nc) as tc:
        with (
            tc.tile_pool(name="sb", bufs=1) as sb,
            tc.tile_pool(name="sbw", bufs=4) as sbw,
            tc.tile_pool(name="ptp", bufs=17) as ptp,
            tc.tile_pool(name="sm", bufs=3) as smp,
            tc.tile_pool(name="ktp", bufs=3) as ktp,
            tc.tile_pool(name="ysp", bufs=6) as ysp,
            tc.tile_pool(name="ps3", bufs=2, space="PSUM") as ps3,
            tc.tile_pool(name="psu", bufs=1, space="PSUM") as psu,
        ):
            # ---------------- persistent SBUF ----------------
            x8_sb = sb.tile([128, 4, 2, KPAD], fp8, name="x8_sb", tag="x8")
            xr_sb = sb.tile([128, 4, 2, KPAD], fp8, name="xr_sb", tag="xr")
            # per head-pair tensor t: partitions p = head 2t + p//64,
            # d = p%64; q groups both hold q8, k groups hold (k_hi, k_lo)
            qdr = [sb.tile([128, S], fp8, name=f"qdr{t}", tag=f"qdr{t}")
                   for t in range(4)]
            kdr = [sb.tile([128, 2, KPAD], fp8, name=f"kdr{t}", tag=f"kdr{t}")
                   for t in range(4)]
            v_sb = sb.tile([128, NKT * VBLK], fp16, name="v_sb", tag="v")
            wv8_sb = sb.tile([128, 4, 2, F], fp8, name="wv8_sb", tag="wv8")
            wvr_sb = sb.tile([128, 4, 2, F], fp8, name="wvr_sb", tag="wvr")
            wo_sb = sb.tile([128, 4, N_STATE], fp16, name="wo_sb", tag="wo")
            WV_sb = sb.tile([128, NKT, F], fp16, name="WV_sb", tag="WV")
            wvT_sb = sb.tile([128, 4, KPAD], fp16, name="wvT_sb", tag="wvT")
            bq_sb = sb.tile([128, 4], f32, name="bq_sb", tag="bq")
            zero_col = sb.tile([128, 1], f32, name="zero_col", tag="zc")
            i32_col = sb.tile([128, 1], f32, name="i32_col", tag="i32")
            id_sb = sb.tile([128, 128], fp16, name="id_sb", tag="iden")
            # out-projection partials (fc0+fc1 [+fc2] pre-chained in-window)
            yp_sb = sb.tile([128, NKT, N_STATE], fp16, name="yp_sb", tag="yp")

            # uacc banks double as the v-projection / U accumulators
            uaccA = psu.tile([128, 512], f32, name="uaccA", tag="uaccA")
            uaccB = psu.tile([128, 512], f32, name="uaccB", tag="uaccB")

            # ---------------- input DMAs ----------------
            # HWDGE dispatch (625ns each) and the DMA engines serialize
            # globally, so transfer priority = emission order and DMA count
            # matters: pair-0 q/k weights ride in two packed tensors, then
            # x seq-chunks (512-col chunks keep the contiguous run at 512B —
            # shorter runs pay the <512B 2x descriptor penalty).
            wqp_sb = sbw.tile([128, 2, 4, 2, 128], fp8, name="wqp_sb",
                              tag="wp", bufs=2)
            wkp_sb = sbw.tile([128, 2, 4, 2, 128], fp8, name="wkp_sb",
                              tag="wp", bufs=2)
            nc.sync.dma_start(out=wqp_sb[:], in_=wqp)
            nc.sync.dma_start(out=wkp_sb[:], in_=wkp)
            wslq8, wslqr = [wqp_sb[:, 0]], [wqp_sb[:, 1]]
            wslk8, wslkr = [wkp_sb[:, 0]], [wkp_sb[:, 1]]
            nc.scalar.dma_start(out=bq_sb[:], in_=bqh)
            for c0 in (0, 512, 1024):
                nc.sync.dma_start(out=x8_sb[:, :, :, c0:c0 + 512],
                                  in_=x8d[:, :, :, c0:c0 + 512])
                nc.sync.dma_start(out=xr_sb[:, :, :, c0:c0 + 512],
                                  in_=xrd[:, :, :, c0:c0 + 512])
            nc.sync.dma_start(out=wv8_sb[:], in_=wv8)
            nc.sync.dma_start(out=wvr_sb[:], in_=wvr)
            nc.vector.memset(zero_col[:], 0.0)
            nc.vector.memset(i32_col[:], 1.0 / WSCALE)
            nc.vector.memset(WV_sb[64:128, NKT - 1, :], 0.0)
            # v ones-columns and k pad columns via memset (no DMA)
            nc.vector.memset(
                v_sb[:].rearrange("p (t h c) -> p t h c",
                                  t=NKT, h=8)[:, :, :, 64:65], 1.0)
            for t in range(4):
                nc.vector.memset(kdr[t][:, :, S:KPAD], 0.0)
            # warm the ACT exp table off the critical path
            warm = smp.tile([128, 1], f32, name="warm", tag="warm")
            nc.scalar.activation(warm[:], zero_col[:], Exp, scale=1.0)

            # ---------------- projections (fp8 DR, 3-term residual) -------
            # one 256-col unit per borrow: allocate a big3 slot, accumulate
            # 12 DR matmuls, stage, release. k staging copies the psum to a
            # f32 scratch (DVE, in-borrow); the fp8 hi/lo split runs on the
            # idle Pool engine off-borrow.
            def proj_unit(qk, t, u, grid=QCH6, stage_dve=False):
                lbl(f"proj_{qk}{t}_u{u}")
                q0, qn = grid[u]
                pacc = ps3.tile([128, 1536], f32, name="pacc", tag="big3")
                w8sl, wrsl = ((wslq8[t], wslqr[t]) if qk == "q"
                              else (wslk8[t], wslkr[t]))
                pairs = [(w8sl, x8_sb), (wrsl, x8_sb), (w8sl, xr_sb)]
                for ti, (wsl, xs) in enumerate(pairs):
                    for g in range(4):
                        mm(out=pacc[:, q0:q0 + qn],
                           lhsT=wsl[:, g, :, :],
                           rhs=xs[:, g, :, q0:q0 + qn],
                           start=(ti == 0 and g == 0),
                           stop=(ti == 2 and g == 3), perf_mode=DR)
                if qk == "q":
                    nc.vector.tensor_scalar_add(
                        out=qdr[t][:, q0:q0 + qn], in0=pacc[:, q0:q0 + qn],
                        scalar1=bq_sb[:, t:t + 1])
                elif stage_dve:
                    nc.vector.tensor_scalar_add(
                        out=kdr[t][:, 0, q0:q0 + qn], in0=pacc[:, q0:q0 + qn],
                        scalar1=zero_col[:, 0:1])
                    nc.vector.tensor_sub(kdr[t][:, 1, q0:q0 + qn],
                                         pacc[:, q0:q0 + qn],
                                         kdr[t][:, 0, q0:q0 + qn])
                else:
                    kt_ = ktp.tile([128, 256], f32, name="kt_", tag="ktmp")
                    nc.vector.tensor_copy(out=kt_[:, 0:qn],
                                          in_=pacc[:, q0:q0 + qn])
                    nc.gpsimd.tensor_copy(out=kdr[t][:, 0, q0:q0 + qn],
                                          in_=kt_[:, 0:qn])
                    nc.gpsimd.tensor_sub(kdr[t][:, 1, q0:q0 + qn],
                                         kt_[:, 0:qn],
                                         kdr[t][:, 0, q0:q0 + qn])

            # ---------------- attention building blocks ----------------
            def st_exp(h, kt):
                lbl(f"st_{h},{kt}")
                t, r = h // 2, 64 * (h % 2)
                stp = ps3.tile([128, 1536], f32, name="stp", tag="big3")
                for q0, qn in QCH:
                    mm(out=stp[:, q0:q0 + qn],
                       lhsT=kdr[t][r:r + 64, :, kt * 128:(kt + 1) * 128],
                       rhs=qdr[t][r:r + 64, q0:q0 + qn][:, None, :
                                                        ].to_broadcast(
                           (64, 2, qn)),
                       start=True, stop=True, perf_mode=DR)
                pt = ptp.tile([128, 1536], fp16, name="pt", tag="pt")
                nc.scalar.activation(pt[:, 0:S], stp[:, 0:S], Exp,
                                     scale=EXP_SCALE)
                return pt

            def u_emit(h, kt, pt, accA=None, accB=None):
                lbl(f"U_{h},{kt}")
                accA = uaccA if accA is None else accA
                accB = uaccB if accB is None else accB
                kn = min(128, S - kt * 128)
                vcol = kt * VBLK + h * 65
                for qt in range(NKT):
                    qn = min(128, S - qt * 128)
                    acc = accA if qt < 6 else accB
                    slot = (qt % 6) * SLOTW
                    mm(out=acc[0:qn, slot:slot + 65],
                       lhsT=pt[0:kn, qt * 128:qt * 128 + qn],
                       rhs=v_sb[0:kn, vcol:vcol + 65],
                       start=False, stop=False, skip_group_check=True)

            def v_emit(sq):
                lbl(f"v_{sq}")
                sn = min(128, S - sq * 128)
                acc = uaccA if sq % 2 == 0 else uaccB
                for ti, (xs, ws) in enumerate(
                        [(x8_sb, wv8_sb), (x8_sb, wvr_sb), (xr_sb, wv8_sb)]):
                    for g in range(4):
                        mm(out=acc[0:sn, 0:512],
                           lhsT=xs[:, g, :, sq * 128:sq * 128 + sn],
                           rhs=ws[:, g, :, :],
                           start=(ti == 0 and g == 0),
                           stop=(ti == 2 and g == 3), perf_mode=DR)
                nc.vector.tensor_scalar_mul(
                    out=v_sb[0:sn, sq * VBLK:(sq + 1) * VBLK].rearrange(
                        "p (h c) -> p h c", h=8)[:, :, 0:64],
                    in0=acc[0:sn, 0:512].rearrange("p (h c) -> p h c", h=8),
                    scalar1=i32_col[0:sn, 0:1])

            def memsets():
                lbl("memsets")
                nc.vector.memset(uaccA[:], 0.0)
                nc.vector.memset(uaccB[:], 0.0)

            def norm_emit(h, accA=None, accB=None):
                lbl(f"norm_{h}")
                accA = uaccA if accA is None else accA
                accB = uaccB if accB is None else accB
                rc = smp.tile([128, 12], f32, name="rc", tag="rc")
                nc.vector.reciprocal(
                    rc[:, 0:6].rearrange("p (s c) -> p s c", c=1),
                    accA[:, 0:6 * SLOTW].rearrange(
                        "p (s c) -> p s c", s=6)[:, :, 64:65])
                nc.vector.reciprocal(
                    rc[:, 6:11].rearrange("p (s c) -> p s c", c=1),
                    accB[:, 0:5 * SLOTW].rearrange(
                        "p (s c) -> p s c", s=5)[:, :, 64:65])
                nc.vector.reciprocal(rc[0:92, 11:12],
                                     accB[0:92, 5 * SLOTW + 64:5 * SLOTW + 65])
                for qt in range(NKT):
                    sn = min(128, S - qt * 128)
                    acc = accA if qt < 6 else accB
                    slot = (qt % 6) * SLOTW
                    nc.vector.tensor_scalar_mul(
                        out=WV_sb[0:sn, qt, h * 64:(h + 1) * 64],
                        in0=acc[0:sn, slot:slot + 64],
                        scalar1=rc[0:sn, qt:qt + 1])

            def transp_emit(p):
                lbl(f"transp_{p}")
                for qt in range(NKT):
                    nc.sync.dma_start(
                        out=wvT_sb[:, p, qt * 128:(qt + 1) * 128],
                        in_=WV_sb[:, qt, p * 128:(p + 1) * 128],
                        transpose=True)

            # out-projection pre-chains: accumulate ready fc blocks for one
            # (sq, ch) unit in a big3 borrow, then flush/add into yp
            OSEQ = [(sq, ch) for sq in range(NKT) for ch in range(2)]
            CHF = {}

            def fc_chain(i, nfc):
                sq, ch = OSEQ[i]
                lbl(f"fch{nfc}_{sq},{ch}")
                sn = min(128, S - sq * 128)
                done = CHF.get((sq, ch), 0)
                reg = ps3.tile([128, 1536], f32, name="fcp", tag="big3")
                for j, fc in enumerate(range(done, nfc)):
                    mm(out=reg[0:sn, 0:512],
                       lhsT=wvT_sb[:, fc, sq * 128:sq * 128 + sn],
                       rhs=wo_sb[:, fc, ch * 512:(ch + 1) * 512],
                       start=(j == 0), stop=(fc == nfc - 1),
                       skip_group_check=True)
                dst = yp_sb[0:sn, sq, ch * 512:(ch + 1) * 512]
                if done == 0:
                    nc.vector.tensor_copy(out=dst, in_=reg[0:sn, 0:512])
                else:
                    nc.vector.tensor_tensor(out=dst, in0=reg[0:sn, 0:512],
                                            in1=dst,
                                            op=mybir.AluOpType.add)
                CHF[(sq, ch)] = nfc

            # ---------------- startup: head 0, kt 0-2, per q-chunk -------
            # exp starts as soon as x chunk 0 + pair-0 weights land; later
            # q-chunks stream in behind their x DMAs.
            KG0 = [(0, 128), (128, 256), (384, 128)]
            proj_unit("q", 0, 0)
            proj_unit("q", 0, 1)
            proj_unit("k", 0, 0, grid=KG0, stage_dve=True)
            pts = {}
            for kt in range(3):
                pts[(0, kt)] = ptp.tile([128, 1536], fp16, name="pt",
                                        tag="pt")

            def stc_mm(stp, kt, ch):
                lbl(f"stm_{kt}c{ch}")
                q0, qn = QCH[ch]
                mm(out=stp[:, kt * 512:kt * 512 + qn],
                   lhsT=kdr[0][0:64, :, kt * 128:(kt + 1) * 128],
                   rhs=qdr[0][0:64, q0:q0 + qn][:, None, :].to_broadcast(
                       (64, 2, qn)),
                   start=True, stop=True, perf_mode=DR)

            def stc_exp(stp, kt, ch):
                lbl(f"stc_{kt}c{ch}")
                q0, qn = QCH[ch]
                nc.scalar.activation(pts[(0, kt)][:, q0:q0 + qn],
                                     stp[:, kt * 512:kt * 512 + qn], Exp,
                                     scale=EXP_SCALE)

            # all three kt matmuls land before the first exp reads the tile
            # (a trailing write would serialize behind the exp's read)
            proj_unit("k", 0, 1, grid=KG0, stage_dve=True)
            stpA = ps3.tile([128, 1536], f32, name="stpA", tag="big3")
            for kt in range(3):
                stc_mm(stpA, kt, 0)
            for kt in range(3):
                stc_exp(stpA, kt, 0)
            proj_unit("q", 0, 2)
            proj_unit("q", 0, 3)
            stpB = ps3.tile([128, 1536], f32, name="stpB", tag="big3")
            for kt in range(3):
                stc_mm(stpB, kt, 1)
            for kt in range(3):
                stc_exp(stpB, kt, 1)
            proj_unit("k", 0, 2, grid=KG0, stage_dve=True)
            proj_unit("q", 0, 4)
            proj_unit("q", 0, 5)
            stpC = ps3.tile([128, 1536], f32, name="stpC", tag="big3")
            for kt in range(3):
                stc_mm(stpC, kt, 2)
            for kt in range(3):
                stc_exp(stpC, kt, 2)

            # head-pair tensors t1-3 + wo on the SP queue: they transfer
            # strictly after the x chunks (needed from window 1 / window 4)
            def load_w3(dram):
                w3 = sbw.tile([128, 3, 4, 2, 128], fp8, name="w3", tag="w3",
                              bufs=4)
                nc.sync.dma_start(
                    out=w3[:], in_=dram[1:4].rearrange("t p g i m -> p t g i m"))
                return [w3[:, t - 1] for t in range(1, 4)]
            wslq8 += load_w3(wq8)
            wslqr += load_w3(wqr)
            wslk8 += load_w3(wk8)
            wslkr += load_w3(wkr)
            nc.sync.dma_start(out=wo_sb[:], in_=wod)
            nc.scalar.dma_start(out=id_sb[:], in_=iden)

            # ---------------- pipelined attention ----------------
            # window h, slot kt: S^T/exp(h, kt), then the scheduled fillers
            FILL = {}

            def add(h, kt, fn):
                FILL.setdefault((h, kt), []).append(fn)

            # w0: remaining k units (256-col, slots 1-4; col (kt+1)*128
            # needed by slot kt) + v tiles (v borrows uacc, no big3 hit)
            for ui, kt in enumerate((1, 2, 3, 4)):
                add(0, kt, lambda u=ui + 2: proj_unit("k", 0, u))
            # v(11) leads w1 slot 0, emitted before the memsets: the uacc
            # WAR/WAW deps then force memset -> U(0) to wait for v's drain
            for sq in range(3, 11):
                add(0, sq - 2, lambda s=sq: v_emit(s))
            add(0, 9, lambda: v_emit(0))
            add(0, 10, lambda: v_emit(1))
            add(0, 11, lambda: v_emit(2))
            add(1, 0, lambda: v_emit(11))
            # w1: U(0) window-lagged + t1 projections (192-col units; k
            # units u0-3 late in w1, u4-7 early in w2 before their kt use)
            add(1, 0, memsets)
            for kt in range(1, NKT):
                add(1, kt, lambda k=kt - 1: u_emit(0, k, pts.pop((0, k))))
            for u in range(12):
                add(1, u, lambda uu=u: proj_unit("q", 1, uu, QCH12))
            for u in range(2):
                add(0, u + 10, lambda uu=u: proj_unit("k", 1, uu, QCH12))
            for u in range(2, 12):
                add(2, u - 2, lambda uu=u: proj_unit("k", 1, uu, QCH12))
            # w2-7: norm(h-2), U(h-1) window-lagged
            # the last U unit of each head slides to the next window's
            # slot 0 (just before that head's norm) so slot 11 carries only
            # one U unit alongside the q filler
            for h in range(2, 7):
                add(h, 0, lambda hh=h - 2: u_emit(hh, 11, pts.pop((hh, 11))))
                add(h, 0, lambda hh=h - 2: norm_emit(hh))
                add(h, 1, memsets)
                add(h, 1, lambda hh=h - 1: u_emit(hh, 0, pts.pop((hh, 0))))
                for kt in range(2, NKT):
                    add(h, kt,
                        lambda hh=h - 1, k=kt - 1: u_emit(hh, k,
                                                          pts.pop((hh, k))))
            # w7: U(6) compressed 2-per-slot into slots 1-6, norm(6) +
            # memsets at slot 7, then U(7, kt<=6) starts in-window on the
            # freed uacc banks — the tail needs no held u2 slot at all
            add(7, 0, lambda: u_emit(5, 11, pts.pop((5, 11))))
            add(7, 0, lambda: norm_emit(5))
            add(7, 1, memsets)
            for kt in range(11):
                add(7, 1 + kt // 2, lambda k=kt: u_emit(6, k,
                                                        pts.pop((6, k))))
            add(7, 7, lambda: u_emit(6, 11, pts.pop((6, 11))))
            add(7, 7, lambda: norm_emit(6))
            add(7, 7, memsets)
            for kt in range(7):
                add(7, 8 + kt // 2, lambda k=kt: u_emit(7, k,
                                                        pts.pop((7, k))))
            # t2 over w3-4, t3 over w5-6: q units fill the odd window (plus
            # the first two k units), the remaining k units run JIT in the
            # even window at slot = their kt - 2
            for t, wq in ((2, 3), (3, 5)):
                for u in range(12):
                    add(wq, u, lambda uu=u, tt=t: proj_unit("q", tt, uu,
                                                            QCH12))
                for u in range(2):
                    add(wq - 1, u + 10, lambda uu=u, tt=t: proj_unit(
                        "k", tt, uu, QCH12))
                for u in range(2, 12):
                    add(wq + 1, u - 2, lambda uu=u, tt=t: proj_unit(
                        "k", tt, uu, QCH12))
            add(3, 2, lambda: transp_emit(0))
            add(5, 2, lambda: transp_emit(1))
            add(7, 1, lambda: transp_emit(2))

            for h in range(8):
                for kt in range(NKT):
                    if (h, kt) not in pts:
                        pts[(h, kt)] = st_exp(h, kt)
                    for fn in FILL.get((h, kt), ()):
                        fn()

            # ---------------- tail ----------------
            # head 7's U accumulates into a spare big3 slot (two clean
            # 512-col bank regions) so norm(6) can drain the uacc banks in
            # parallel; U(7) runs qt-outer with per-qt normalize + transpose
            # so the pair-3 transposes trickle out as early as possible
            lbl("tail_u2")
            bigA = ps3.tile([128, 1536], f32, name="bigA", tag="big3")
            bigB = ps3.tile([128, 1536], f32, name="bigB", tag="big3")
            regions = ([(bigA, 512 * j) for j in range(3)]
                       + [(bigB, 512 * j) for j in range(3)]
                       + [(uaccA, 0), (uaccB, 0)])
            OPENED = {}

            def fc_open(i):
                # pre-open a unit's fc0-2 accumulation so the final loop
                # only adds fc3 + copyout
                sq, ch = OSEQ[i]
                lbl(f"fcopen_{sq},{ch}")
                sn = min(128, S - sq * 128)
                reg, off = regions[i % 8]
                for fc in range(3):
                    mm(out=reg[0:sn, off:off + 512],
                       lhsT=wvT_sb[:, fc, sq * 128:sq * 128 + sn],
                       rhs=wo_sb[:, fc, ch * 512:(ch + 1) * 512],
                       start=(fc == 0), stop=False, skip_group_check=True)
                OPENED[i] = (reg, off)

            for kt in range(7, NKT):
                u_emit(7, kt, pts.pop((7, kt)))
                if kt >= 9:
                    fc_open(2 * (kt - 9))
                    fc_open(2 * (kt - 9) + 1)
            fc_open(4)
            fc_open(5)
            # batched reciprocals (norm_emit-style), then per-qt muls +
            # pair-3 transposes; all tail DMAs ride the idle SP queue — an
            # ACT-queue dispatch (667ns on ACT.SEQ) would stall the next
            # ACT copyout issue
            rc7 = smp.tile([128, 12], f32, name="rc7", tag="rc")
            nc.vector.reciprocal(
                rc7[:, 0:6].rearrange("p (s c) -> p s c", c=1),
                uaccA[:, 0:6 * SLOTW].rearrange(
                    "p (s c) -> p s c", s=6)[:, :, 64:65])
            nc.vector.reciprocal(
                rc7[:, 6:11].rearrange("p (s c) -> p s c", c=1),
                uaccB[:, 0:5 * SLOTW].rearrange(
                    "p (s c) -> p s c", s=5)[:, :, 64:65])
            nc.vector.reciprocal(rc7[0:92, 11:12],
                                 uaccB[0:92, 5 * SLOTW + 64:5 * SLOTW + 65])
            for qt in range(NKT):
                qn = min(128, S - qt * 128)
                acc = uaccA if qt < 6 else uaccB
                slot = (qt % 6) * SLOTW
                nc.vector.tensor_scalar_mul(
                    out=WV_sb[0:qn, qt, 7 * 64:8 * 64],
                    in0=acc[0:qn, slot:slot + 64],
                    scalar1=rc7[0:qn, qt:qt + 1])
                nc.sync.dma_start(
                    out=wvT_sb[:, 3, qt * 128:(qt + 1) * 128],
                    in_=WV_sb[:, qt, 384:512],
                    transpose=True)
            # out-projection tail: fc0/fc1 sit in yp; per unit the
            # remaining fc blocks accumulate into one of 8 rotating
            # 512-col psum regions, yp folds in via an identity matmul
            # (a PE-side add), copy-out alternates ACT/DVE, y per q-tile
            # y staging lives in yp (contiguous across sq) so y can go out
            # in 5 batched DMAs instead of 12 dispatch-serialized ones
            YGRP = [(0, 3), (3, 3), (6, 3), (9, 2), (11, 1)]
            for i, (sq, ch) in enumerate(OSEQ):
                lbl(f"outproj_{sq},{ch}")
                sn = min(128, S - sq * 128)
                # units 6-11 recycle the early big3 regions (their first
                # users finish fast); the uacc banks join the rotation only
                # from unit 12, after norm(7) has drained them
                reg, off = regions[i if i < 6 else (i - 6) % 8]
                done = CHF.get((sq, ch), 0)
                first = range(done, 4)[0] if i not in OPENED else 3
                for fc in range(first, 4):
                    mm(out=reg[0:sn, off:off + 512],
                       lhsT=wvT_sb[:, fc, sq * 128:sq * 128 + sn],
                       rhs=wo_sb[:, fc, ch * 512:(ch + 1) * 512],
                       start=(fc == first and i not in OPENED),
                       stop=(done == 0 and fc == 3),
                       skip_group_check=True)
                if done:
                    mm(out=reg[0:sn, off:off + 512],
                       lhsT=id_sb[0:sn, 0:sn],
                       rhs=yp_sb[0:sn, sq, ch * 512:(ch + 1) * 512],
                       start=False, stop=True, skip_group_check=True)
                dst = yp_sb[0:sn, sq, ch * 512:(ch + 1) * 512]
                # DVE is busy with norm(6)/norm(7) at tail start, so the
                # first copyouts all ride ACT
                if i < 6 or i % 2 == 0:
                    nc.scalar.activation(dst, reg[0:sn, off:off + 512],
                                         Copy, scale=1.0)
                else:
                    nc.vector.tensor_copy(out=dst,
                                          in_=reg[0:sn, off:off + 512])
                if ch == 1:
                    for g0, gn in YGRP:
                        if sq == g0 + gn - 1:
                            rows = min(S, (g0 + gn) * 128) - g0 * 128
                            nc.sync.dma_start(
                                out=y[g0 * 128:g0 * 128 + rows, :].rearrange(
                                    "(t p) c -> p t c", p=128)
                                if gn > 1 else y[g0 * 128:g0 * 128 + rows, :],
                                in_=yp_sb[:, g0:g0 + gn, :]
                                if gn > 1 else yp_sb[0:rows, sq, :])

    nc.compile()
    return nc


def get_nc():
    if "nc" not in _CACHE:
        _CACHE["nc"] = _build()
    return _CACHE["nc"]


def make_in_maps(x, Wq, bq, Wk, Wv, bv, Wo, bo):
    import ml_dtypes
    e4 = ml_dtypes.float8_e4m3fn
    f16 = np.float16

    x = np.asarray(x, dtype=np.float32)
    Wq = np.asarray(Wq, dtype=np.float32)
    Wk = np.asarray(Wk, dtype=np.float32)
    Wv = np.asarray(Wv, dtype=np.float32)
    Wo = np.asarray(Wo, dtype=np.float32)
    bq = np.asarray(bq, dtype=np.float32)

    def wsplit(Wc):
        # Wc: [512 feat, 1024 state] scaled; -> (hi, lo) each
        # [4, 128(p), 4(g), 2(i), 128(m)] fp8 with W[128t+m, (2g+i)*128+p]
        W8 = Wc.astype(e4)
        Wr = (Wc - W8.astype(np.float32)).astype(e4)
        out = []
        for Wx in (W8, Wr):
            a = Wx.T.reshape(4, 2, 128, 4, 128)     # [g, i, p, t, m]
            out.append(np.ascontiguousarray(a.transpose(3, 2, 0, 1, 4)))
        return out

    def vsplit(Wc):
        # -> [128(p), 4(g), 2(i), 512(f)] fp8
        W8 = Wc.astype(e4)
        Wr = (Wc - W8.astype(np.float32)).astype(e4)
        out = []
        for Wx in (W8, Wr):
            a = Wx.T.reshape(4, 2, 128, F)          # [g, i, p, f]
            out.append(np.ascontiguousarray(a.transpose(2, 0, 1, 3)))
        return out

    in_maps = []
    for c in range(NCORES):
        b, h2 = divmod(c, 2)
        sl = slice(h2 * F, (h2 + 1) * F)
        xT = np.zeros((N_STATE, KPAD), dtype=np.float32)
        xT[:, 0:S] = x[b].T
        x8f = xT.astype(e4)
        xrf = (xT - x8f.astype(np.float32)).astype(e4)
        x8 = x8f.reshape(4, 2, 128, KPAD).transpose(2, 0, 1, 3)
        xr = xrf.reshape(4, 2, 128, KPAD).transpose(2, 0, 1, 3)
        wq8_, wqr_ = wsplit(Wq[sl] * WSCALE)
        wk8_, wkr_ = wsplit(Wk[sl] * WSCALE)
        wv8_, wvr_ = vsplit(Wv[sl] * WSCALE)
        wod_ = Wo[:, sl].T.reshape(4, 128, N_STATE).transpose(1, 0, 2)
        iden_ = np.eye(128, dtype=f16)
        # feature f = 128t + p  ->  bqh[p, t]
        bqh_ = (WSCALE * bq[sl]).reshape(4, 128).T
        wqp_ = np.ascontiguousarray(np.stack([wq8_[0], wqr_[0]], axis=1))
        wkp_ = np.ascontiguousarray(np.stack([wk8_[0], wkr_[0]], axis=1))
        in_maps.append(dict(
            x8d=np.ascontiguousarray(x8), xrd=np.ascontiguousarray(xr),
            wq8=wq8_, wqr=wqr_, wk8=wk8_, wkr=wkr_,
            wqp=wqp_, wkp=wkp_,
            wv8=wv8_, wvr=wvr_,
            wod=np.ascontiguousarray(wod_.astype(f16)), iden=iden_,
            bqh=np.ascontiguousarray(bqh_, dtype=np.float32),
        ))
    return in_maps


def kernel(x, Wq, bq, Wk, Wv, bv, Wo, bo):
    global LAST_RESULTS
    from concourse.bass_utils import run_bass_kernel_spmd

    try:
        import antenv.axon_hooks  # noqa: F401
    except ImportError:
        os.environ["BASS_NEVER_TRACE"] = "1"

    nc = get_nc()
    in_maps = make_in_maps(x, Wq, bq, Wk, Wv, bv, Wo, bo)
    res = run_bass_kernel_spmd(nc, in_maps, list(range(NCORES)))
    LAST_RESULTS = res
    Wo32 = np.asarray(Wo, dtype=np.float32)
    extra = (Wo32 @ np.asarray(bv, dtype=np.float32)
             + np.asarray(bo, dtype=np.float32))
    out = np.stack([res.results[2 * b]["y"].astype(np.float32)
                    + res.results[2 * b + 1]["y"].astype(np.float32)
                    + extra[None, :] for b in range(B)])
    return out.astype(np.float32)
